# revision 9
# baseline (speedup 1.0000x reference)
"""Distributed causal attention head for TRN2 (8 NeuronCores), v3.

Problem: B=4, S=4096, D=1024, H=64 fp32.
  q,k,v = x @ W{q,k,v}; scores = q k^T / sqrt(H); causal softmax; out = P v.

Sharding (fully SPMD-uniform, one NEFF for all 8 cores):
  - 4 batches x 2 cores per batch (pair replica groups [[0,1],[2,3],[4,5],[6,7]]).
  - Within a pair the KEY dimension is split by interleaved 128-row chunks:
    core g owns global key chunks {2i+g}. Host pre-transposes each core's
    2048 input rows to x^T [1024, 2048] bf16 so all DMA loads are linear
    (2 KB per-partition runs). Weights/masks are host-relaid partition-major.
  - Q^T is pair-AllGathered in TWO pieces (local key-chunk halves) so the
    first attention blocks can start while the second AllGather is in flight;
    the natural-V projection runs inside the AllGather latency window.
  - Per-core partial (numerator | denominator) = [65, 4096] fp32 goes straight
    to DRAM; the HOST merges the pair (add), divides, and transposes. No
    ReduceScatter on device.

Compute layout:
  - k|q packed projection contracts on the partition dim, x^T tiles moving
    (N=1024); V is computed in natural [keys, 64] layout with x^T chunks
    stationary (no PE transposes anywhere in the kernel).
  - Scores transposed with 2x PE row tiling (64-contract): tile (0,0) does
    local chunks 0..t of q-block t, tile (64,0) chunks t+1..2t+1. kT/qT are
    duplicated into both SBUF partition halves (SBUF->SBUF DMA).
  - Gathered Q^T stays source-major [128, 2, 16, 128]; score matmuls use two
    N=256 strided-AP pieces so no interleave scatter-DMA is needed.
  - exp on the scalar engine over 3-bank PSUM score sets (N=1536, 24 uniform
    sets of 3 chunks), scale=1/8 fused, bf16 out. AV matmuls are emitted
    per-set right after each exp (event-driven) to avoid PE bursts.
  - V is augmented with a ones column so the AV matmul also produces the
    softmax denominator (row 64 of the [65, 512] accumulator).
"""

import sys

sys.path.insert(0, "/opt/trn_rl_repo")

import numpy as np
import ml_dtypes

B, S, D, H = 4, 4096, 1024, 64
RPC = S // 2            # rows (keys/queries) owned per core
QB = 512                # query block width
NQB = S // QB           # 8 query blocks
NKC = RPC // 128        # 16 local key chunks
HKC = NKC // 2          # chunks per AllGather half
BF16 = ml_dtypes.bfloat16
PAIRS = [[0, 1], [2, 3], [4, 5], [6, 7]]

_CACHE = {}


def _build():
    import concourse.bass as bass
    import concourse.mybir as mybir
    from concourse import bacc, tile
    from concourse.bass import ts

    f32 = mybir.dt.float32
    bf16 = mybir.dt.bfloat16
    Alu = mybir.AluOpType
    Act = mybir.ActivationFunctionType

    nc = bacc.Bacc(None, target_bir_lowering=False)

    x_ext = nc.declare_dram_parameter("x", [D, RPC], bf16, isOutput=False)
    wkq_ext = nc.declare_dram_parameter("wkq", [128, 8 * 128], bf16, isOutput=False)
    wv_ext = nc.declare_dram_parameter("wv", [128, 8 * H], bf16, isOutput=False)
    mask_ext = nc.declare_dram_parameter("mask", [128, 2 * QB], bf16, isOutput=False)
    out_ext = nc.declare_dram_parameter("out", [H + 1, S], f32, isOutput=True)

    with tile.TileContext(nc) as tc:
        with (
            tc.tile_pool(name="persist", bufs=1) as persist,
            tc.tile_pool(name="dram", bufs=1, space="DRAM") as dram,
        ):
            # --- persistent SBUF tensors ---
            xT = persist.tile([128, 8, RPC], bf16, tag="xT")
            wkq_sb = persist.tile([128, 8, 128], bf16, tag="wkq")
            wv_sb = persist.tile([128, 8, H], bf16, tag="wv")
            mask_sb = persist.tile([128, 2, QB], bf16, tag="mask")
            kT2 = persist.tile([128, NKC, 128], bf16, tag="kT2")
            qT2g = persist.tile([128, 2, NKC, 128], bf16, tag="qT2g")
            v_all = persist.tile([128, NKC, H + 2], bf16, tag="v_all")
            qtmp = persist.tile([128, RPC], bf16, tag="qtmp")
            zjunk = persist.tile([128, 8], f32, tag="zjunk")
            ejunk = persist.tile([128, 8], bf16, tag="ejunk")

            # preload the exp activation table set early (it costs ~2.7us)
            nc.vector.memset(zjunk[:], 0.0)
            nc.scalar.activation(ejunk[:], zjunk[:], Act.Exp)
            nc.vector.memset(v_all[:, :, H], 1.0)

            # small contiguous weight/mask loads first, then the bulk x halves
            nc.sync.dma_start(out=wkq_sb[:], in_=wkq_ext[:])
            nc.sync.dma_start(out=wv_sb[:], in_=wv_ext[:])
            nc.sync.dma_start(out=mask_sb[:], in_=mask_ext[:])
            for h in range(2):
                for dc in range(8):
                    nc.sync.dma_start(
                        out=xT[:, dc, ts(h, RPC // 2)],
                        in_=x_ext[ts(dc, 128), ts(h, RPC // 2)],
                    )

            q_bounce = []
            q_gath = []
            for h in range(2):
                qb = dram.tile([64, RPC // 2], bf16, tag=f"q_bounce{h}")
                qg = dram.tile([2, 64, RPC // 2], bf16, tag=f"q_gath{h}")
                q_bounce.append(qb)
                q_gath.append(qg)

            # --- phase 1: k|q projections per column half + early AllGathers ---
            with (
                tc.tile_pool(name="pj", bufs=2, space="PSUM") as pj_pool,
                tc.tile_pool(name="pv", bufs=2, space="PSUM") as pv_pool,
            ):
                for h in range(2):
                    kq_ps = pj_pool.tile([128, RPC // 2], f32, tag="kq")
                    for pp in range(2):
                        for dc in range(8):
                            nc.tensor.matmul(
                                kq_ps[:, ts(pp, QB)],
                                lhsT=wkq_sb[:, dc, :],
                                rhs=xT[:, dc, h * (RPC // 2) + pp * QB : h * (RPC // 2) + (pp + 1) * QB],
                                start=(dc == 0),
                                stop=(dc == 7),
                            )
                    nc.vector.tensor_copy(
                        qtmp[64:128, ts(h, RPC // 2)], kq_ps[64:128, :]
                    )
                    nc.sync.dma_start(
                        out=q_bounce[h][:], in_=qtmp[64:128, ts(h, RPC // 2)]
                    )
                    nc.gpsimd.collective_compute(
                        "AllGather",
                        Alu.bypass,
                        replica_groups=PAIRS,
                        ins=[q_bounce[h].opt()],
                        outs=[q_gath[h].opt()],
                    )
                    for kc in range(HKC):
                        nc.any.tensor_copy(
                            kT2[0:64, HKC * h + kc, :], kq_ps[0:64, ts(kc, 128)]
                        )

                # --- natural-layout V inside the AllGather latency window ---
                for kc in range(NKC):
                    v_ps = pv_pool.tile([128, H], f32, tag="v")
                    for dc in range(8):
                        nc.tensor.matmul(
                            v_ps[:],
                            lhsT=xT[:, dc, ts(kc, 128)],
                            rhs=wv_sb[:, dc, :],
                            start=(dc == 0),
                            stop=(dc == 7),
                        )
                    nc.any.tensor_copy(v_all[:, kc, 0:H], v_ps[:])

            # kT high-half duplicate (SBUF->SBUF, partition shift)
            nc.sync.dma_start(out=kT2[64:128, :, :], in_=kT2[0:64, :, :])
            # gathered q into both partition halves, source-major layout
            for h in range(2):
                for src in range(2):
                    nc.sync.dma_start(
                        out=qT2g[0:64, src, ts(h, HKC), :], in_=q_gath[h][src]
                    )
                    nc.sync.dma_start(
                        out=qT2g[64:128, src, ts(h, HKC), :], in_=q_gath[h][src]
                    )

            # --- phase 2: attention ---
            with (
                tc.tile_pool(name="st", bufs=2, space="PSUM") as st_pool,
                tc.tile_pool(name="av", bufs=2, space="PSUM") as av_pool,
                tc.tile_pool(name="p", bufs=3) as p_pool,
                tc.tile_pool(name="o", bufs=3) as o_pool,
            ):
                # Schraudolph exp-approximation constants for the DVE path:
                # bf16 bits of exp(s/8) ~= uint16(s * SCH_A + SCH_B)
                SCH_C = 486411
                SCH_A = 0.125 * float(1 << 23) / float(np.log(2.0)) / 65536.0
                SCH_B = float((127 << 23) - SCH_C) / 65536.0
                u16 = mybir.dt.uint16

                gamma = 0
                cur = None
                av_tiles = {}

                def flush_set(rec):
                    stt, pt = rec["st"], rec["p"]
                    # positions 0-1: true exp on ScalarE; position 2: DVE bit-trick
                    nc.scalar.activation(
                        pt[:, 0:2, :], stt[:, 0:2, :], Act.Exp, scale=0.125
                    )
                    nc.vector.tensor_scalar(
                        pt[:, 2, :].bitcast(u16),
                        stt[:, 2, :],
                        SCH_A,
                        SCH_B,
                        Alu.mult,
                        Alu.add,
                    )
                    for pp, j in rec["masks"]:
                        nc.vector.tensor_tensor(
                            pt[:, pp, :], pt[:, pp, :], mask_sb[:, j, :], Alu.mult
                        )
                    for tt, cc, pp in rec["chunks"]:
                        if cc == 0:
                            av_new = av_pool.tile([H + 1, QB], f32, tag="av")
                            av_tiles[tt] = av_new
                        nc.tensor.matmul(
                            av_tiles[tt][:],
                            lhsT=v_all[:, cc, 0 : H + 1],
                            rhs=pt[:, pp, :],
                            start=(cc == 0),
                            stop=(cc == 2 * tt + 1),
                        )
                        if cc == 2 * tt + 1:
                            o = o_pool.tile([H + 1, QB], f32, tag="o")
                            nc.vector.tensor_copy(o[:], av_tiles[tt][:])
                            nc.sync.dma_start(out=out_ext[:, ts(tt, QB)], in_=o[:])
                            del av_tiles[tt]

                for t in range(NQB):
                    E = 2 * (t + 1)
                    for s in range(t + 1):
                        slot = []
                        completed = []
                        for hh in (0, 1):
                            cid = s if hh == 0 else t + 1 + s
                            if cur is None:
                                st_new = st_pool.tile([128, 3, QB], f32, tag="st")
                                p_new = p_pool.tile([128, 3, QB], bf16, tag="p")
                                cur = {
                                    "st": st_new,
                                    "p": p_new,
                                    "chunks": [],
                                    "masks": [],
                                }
                            pos = gamma % 3
                            slot.append((hh, cid, cur["st"], pos))
                            cur["chunks"].append((t, cid, pos))
                            if cid >= E - 2:
                                cur["masks"].append((pos, cid - (E - 2)))
                            gamma += 1
                            if gamma % 3 == 0:
                                completed.append(cur)
                                cur = None
                        # T0/T8 pieces interleaved so the row tiles run coupled
                        for pc in (0, 1):
                            for hh, cid, stt, pos in slot:
                                nc.tensor.matmul(
                                    stt[:, pos, ts(pc, 256)],
                                    lhsT=kT2[64 * hh : 64 * hh + 64, cid, :],
                                    rhs=qT2g[64 * hh : 64 * hh + 64, :, 2 * t + pc, :],
                                    start=True,
                                    stop=True,
                                    tile_position=(64 * hh, 0),
                                )
                        for rec in completed:
                            flush_set(rec)

    nc.finalize()
    return nc


def _make_masks(g: int) -> np.ndarray:
    # mask[j][kk, qq] = 1 if query (512t + qq) >= key 128*(4t + 2j + g) + kk
    m = np.zeros((2, 128, QB), dtype=np.float32)
    for j in range(2):
        dk = 128 * (2 * j + g) + np.arange(128)[:, None]
        dq = np.arange(QB)[None, :]
        m[j] = (dq >= dk).astype(np.float32)
    return m.astype(BF16)


def _shard_inputs(input, Wq, Wk, Wv):
    x = np.asarray(input)
    wkq = np.concatenate([Wk, Wq], axis=1).astype(np.float32)  # [D, 128]
    # partition-major relayout: wkq_h[p, dc*128+j] = wkq[dc*128+p, j]
    wkq_h = np.ascontiguousarray(
        wkq.reshape(8, 128, 128).transpose(1, 0, 2).reshape(128, 8 * 128)
    ).astype(BF16)
    wv_h = np.ascontiguousarray(
        np.asarray(Wv, dtype=np.float32).reshape(8, 128, H).transpose(1, 0, 2).reshape(128, 8 * H)
    ).astype(BF16)
    masks = []
    for g in range(2):
        m = _make_masks(g)  # [2, 128, QB]
        masks.append(np.ascontiguousarray(m.transpose(1, 0, 2).reshape(128, 2 * QB)))
    in_maps = []
    for c in range(8):
        b, g = c // 2, c % 2
        xs = x[b].reshape(S // 128, 128, D)[g::2].reshape(RPC, D)
        xT = np.ascontiguousarray(xs.T).astype(BF16)
        in_maps.append({"x": xT, "wkq": wkq_h, "wv": wv_h, "mask": masks[g]})
    return in_maps


def _unshard(results):
    out = np.empty((B, S, H), dtype=np.float32)
    for b in range(B):
        merged = results[2 * b]["out"] + results[2 * b + 1]["out"]
        out[b] = (merged[:H] / merged[H : H + 1]).T
    return out


def _run(inputs, trace=False):
    from concourse.bass_utils import run_bass_kernel_spmd

    if "nc" not in _CACHE:
        _CACHE["nc"] = _build()
    nc = _CACHE["nc"]
    in_maps = _shard_inputs(**inputs)
    res = run_bass_kernel_spmd(nc, in_maps, core_ids=list(range(8)), trace=trace)
    out = _unshard(res.results)
    return out, res


def kernel(**inputs) -> np.ndarray:
    out, _ = _run(inputs, trace=False)
    return out


# revision 12
# speedup vs baseline: 1.0649x; 1.0649x over previous
"""Distributed causal attention head for TRN2 (8 NeuronCores), v3.

Problem: B=4, S=4096, D=1024, H=64 fp32.
  q,k,v = x @ W{q,k,v}; scores = q k^T / sqrt(H); causal softmax; out = P v.

Sharding (fully SPMD-uniform, one NEFF for all 8 cores):
  - 4 batches x 2 cores per batch (pair replica groups [[0,1],[2,3],[4,5],[6,7]]).
  - Within a pair the KEY dimension is split by interleaved 128-row chunks:
    core g owns global key chunks {2i+g}. Host pre-transposes each core's
    2048 input rows to x^T [1024, 2048] bf16 so all DMA loads are linear
    (2 KB per-partition runs). Weights/masks are host-relaid partition-major.
  - Q^T is pair-AllGathered in TWO pieces (local key-chunk halves) so the
    first attention blocks can start while the second AllGather is in flight;
    the natural-V projection runs inside the AllGather latency window.
  - Per-core partial (numerator | denominator) = [65, 4096] fp32 goes straight
    to DRAM; the HOST merges the pair (add), divides, and transposes. No
    ReduceScatter on device.

Compute layout:
  - k|q packed projection contracts on the partition dim, x^T tiles moving
    (N=1024); V is computed in natural [keys, 64] layout with x^T chunks
    stationary (no PE transposes anywhere in the kernel).
  - Scores transposed with 2x PE row tiling (64-contract): tile (0,0) does
    local chunks 0..t of q-block t, tile (64,0) chunks t+1..2t+1. kT/qT are
    duplicated into both SBUF partition halves (SBUF->SBUF DMA).
  - Gathered Q^T stays source-major [128, 2, 16, 128]; score matmuls use two
    N=256 strided-AP pieces so no interleave scatter-DMA is needed.
  - exp on the scalar engine over 3-bank PSUM score sets (N=1536, 24 uniform
    sets of 3 chunks), scale=1/8 fused, bf16 out. AV matmuls are emitted
    per-set right after each exp (event-driven) to avoid PE bursts.
  - V is augmented with a ones column so the AV matmul also produces the
    softmax denominator (row 64 of the [65, 512] accumulator).
"""

import sys

sys.path.insert(0, "/opt/trn_rl_repo")

import numpy as np
import ml_dtypes

B, S, D, H = 4, 4096, 1024, 64
RPC = S // 2            # rows (keys/queries) owned per core
QB = 512                # query block width
NQB = S // QB           # 8 query blocks
NKC = RPC // 128        # 16 local key chunks
HKC = NKC // 2          # chunks per AllGather half
BF16 = ml_dtypes.bfloat16
PAIRS = [[0, 1], [2, 3], [4, 5], [6, 7]]

_CACHE = {}


def _build():
    import concourse.bass as bass
    import concourse.mybir as mybir
    from concourse import bacc, tile
    from concourse.bass import ts

    f32 = mybir.dt.float32
    bf16 = mybir.dt.bfloat16
    Alu = mybir.AluOpType
    Act = mybir.ActivationFunctionType

    nc = bacc.Bacc(None, target_bir_lowering=False)

    x_ext = nc.declare_dram_parameter("x", [D, RPC], bf16, isOutput=False)
    wkq_ext = nc.declare_dram_parameter("wkq", [128, 8 * 128], bf16, isOutput=False)
    wv_ext = nc.declare_dram_parameter("wv", [128, 8 * H], bf16, isOutput=False)
    mask_ext = nc.declare_dram_parameter("mask", [128, 2 * QB], bf16, isOutput=False)
    out_ext = nc.declare_dram_parameter("out", [H + 1, S], f32, isOutput=True)

    with tile.TileContext(nc) as tc:
        with (
            tc.tile_pool(name="persist", bufs=1) as persist,
            tc.tile_pool(name="dram", bufs=1, space="DRAM") as dram,
        ):
            # --- persistent SBUF tensors ---
            xT = persist.tile([128, 8, RPC], bf16, tag="xT")
            wkq_sb = persist.tile([128, 8, 128], bf16, tag="wkq")
            wv_sb = persist.tile([128, 8, H], bf16, tag="wv")
            mask_sb = persist.tile([128, 2, QB], bf16, tag="mask")
            kT2 = persist.tile([128, NKC, 128], bf16, tag="kT2")
            qT2g = persist.tile([128, 2, NKC, 128], bf16, tag="qT2g")
            v_all = persist.tile([128, NKC, H + 2], bf16, tag="v_all")
            qtmp = persist.tile([128, RPC], bf16, tag="qtmp")
            zjunk = persist.tile([128, 8], f32, tag="zjunk")
            ejunk = persist.tile([128, 8], bf16, tag="ejunk")

            # preload the exp activation table set early (it costs ~2.7us)
            nc.vector.memset(zjunk[:], 0.0)
            nc.scalar.activation(ejunk[:], zjunk[:], Act.Exp)
            nc.vector.memset(v_all[:, :, H], 1.0)

            # small contiguous weight/mask loads first, then the bulk x halves
            nc.sync.dma_start(out=wkq_sb[:], in_=wkq_ext[:])
            nc.sync.dma_start(out=wv_sb[:], in_=wv_ext[:])
            nc.sync.dma_start(out=mask_sb[:], in_=mask_ext[:])
            for h in range(2):
                for dc in range(8):
                    nc.sync.dma_start(
                        out=xT[:, dc, ts(h, RPC // 2)],
                        in_=x_ext[ts(dc, 128), ts(h, RPC // 2)],
                    )

            q_bounce = []
            q_gath = []
            for h in range(2):
                qb = dram.tile([64, RPC // 2], bf16, tag=f"q_bounce{h}")
                qg = dram.tile([2, 64, RPC // 2], bf16, tag=f"q_gath{h}")
                q_bounce.append(qb)
                q_gath.append(qg)

            # --- phase 1: k|q projections per column half + early AllGathers ---
            with (
                tc.tile_pool(name="pj", bufs=2, space="PSUM") as pj_pool,
                tc.tile_pool(name="pv", bufs=2, space="PSUM") as pv_pool,
            ):
                for h in range(2):
                    kq_ps = pj_pool.tile([128, RPC // 2], f32, tag="kq")
                    for pp in range(2):
                        for dc in range(8):
                            nc.tensor.matmul(
                                kq_ps[:, ts(pp, QB)],
                                lhsT=wkq_sb[:, dc, :],
                                rhs=xT[:, dc, h * (RPC // 2) + pp * QB : h * (RPC // 2) + (pp + 1) * QB],
                                start=(dc == 0),
                                stop=(dc == 7),
                            )
                    nc.vector.tensor_copy(
                        qtmp[64:128, ts(h, RPC // 2)], kq_ps[64:128, :]
                    )
                    nc.scalar.dma_start(
                        out=q_bounce[h][:], in_=qtmp[64:128, ts(h, RPC // 2)]
                    )
                    nc.gpsimd.collective_compute(
                        "AllGather",
                        Alu.bypass,
                        replica_groups=PAIRS,
                        ins=[q_bounce[h].opt()],
                        outs=[q_gath[h].opt()],
                    )
                    for kc in range(HKC):
                        nc.any.tensor_copy(
                            kT2[0:64, HKC * h + kc, :], kq_ps[0:64, ts(kc, 128)]
                        )

                # --- natural-layout V inside the AllGather latency window ---
                for kc in range(NKC):
                    v_ps = pv_pool.tile([128, H], f32, tag="v")
                    for dc in range(8):
                        nc.tensor.matmul(
                            v_ps[:],
                            lhsT=xT[:, dc, ts(kc, 128)],
                            rhs=wv_sb[:, dc, :],
                            start=(dc == 0),
                            stop=(dc == 7),
                        )
                    nc.any.tensor_copy(v_all[:, kc, 0:H], v_ps[:])

            # kT high-half duplicate (SBUF->SBUF, partition shift)
            nc.scalar.dma_start(out=kT2[64:128, :, :], in_=kT2[0:64, :, :])
            # gathered q into both partition halves, source-major layout
            for h in range(2):
                for src in range(2):
                    nc.scalar.dma_start(
                        out=qT2g[0:64, src, ts(h, HKC), :], in_=q_gath[h][src]
                    )
                    nc.scalar.dma_start(
                        out=qT2g[64:128, src, ts(h, HKC), :], in_=q_gath[h][src]
                    )

            # --- phase 2: attention ---
            with (
                tc.tile_pool(name="st", bufs=2, space="PSUM") as st_pool,
                tc.tile_pool(name="av", bufs=2, space="PSUM") as av_pool,
                tc.tile_pool(name="p", bufs=3) as p_pool,
                tc.tile_pool(name="o", bufs=3) as o_pool,
            ):
                # Schraudolph exp-approximation constants for the DVE path:
                # bf16 bits of exp(s/8) ~= uint16(s * SCH_A + SCH_B)
                SCH_C = 486411
                SCH_A = 0.125 * float(1 << 23) / float(np.log(2.0)) / 65536.0
                SCH_B = float((127 << 23) - SCH_C) / 65536.0
                u16 = mybir.dt.uint16

                gamma = 0
                cur = None
                av_tiles = {}

                def flush_set(rec):
                    stt, pt = rec["st"], rec["p"]
                    # position 2: DVE bit-trick first (its AV comes last anyway),
                    # then positions 0-1 as true exp on ScalarE
                    nc.vector.tensor_scalar(
                        pt[:, 2, :].bitcast(u16),
                        stt[:, 2, :],
                        SCH_A,
                        SCH_B,
                        Alu.mult,
                        Alu.add,
                    )
                    nc.scalar.activation(
                        pt[:, 0:2, :], stt[:, 0:2, :], Act.Exp, scale=0.125
                    )
                    for pp, j in rec["masks"]:
                        nc.vector.tensor_tensor(
                            pt[:, pp, :], pt[:, pp, :], mask_sb[:, j, :], Alu.mult
                        )
                    for tt, cc, pp in rec["chunks"]:
                        if cc == 0:
                            av_new = av_pool.tile([H + 1, QB], f32, tag="av")
                            av_tiles[tt] = av_new
                        nc.tensor.matmul(
                            av_tiles[tt][:],
                            lhsT=v_all[:, cc, 0 : H + 1],
                            rhs=pt[:, pp, :],
                            start=(cc == 0),
                            stop=(cc == 2 * tt + 1),
                        )
                        if cc == 2 * tt + 1:
                            o = o_pool.tile([H + 1, QB], f32, tag="o")
                            nc.vector.tensor_copy(o[:], av_tiles[tt][:])
                            nc.sync.dma_start(out=out_ext[:, ts(tt, QB)], in_=o[:])
                            del av_tiles[tt]

                for t in range(NQB):
                    E = 2 * (t + 1)
                    for s in range(t + 1):
                        slot = []
                        completed = []
                        for hh in (0, 1):
                            cid = s if hh == 0 else t + 1 + s
                            if cur is None:
                                st_new = st_pool.tile([128, 3, QB], f32, tag="st")
                                p_new = p_pool.tile([128, 3, QB], bf16, tag="p")
                                cur = {
                                    "st": st_new,
                                    "p": p_new,
                                    "chunks": [],
                                    "masks": [],
                                }
                            pos = gamma % 3
                            slot.append((hh, cid, cur["st"], pos))
                            cur["chunks"].append((t, cid, pos))
                            if cid >= E - 2:
                                cur["masks"].append((pos, cid - (E - 2)))
                            gamma += 1
                            if gamma % 3 == 0:
                                completed.append(cur)
                                cur = None
                        # T0/T8 pieces interleaved so the row tiles run coupled
                        for pc in (0, 1):
                            for hh, cid, stt, pos in slot:
                                nc.tensor.matmul(
                                    stt[:, pos, ts(pc, 256)],
                                    lhsT=kT2[64 * hh : 64 * hh + 64, cid, :],
                                    rhs=qT2g[64 * hh : 64 * hh + 64, :, 2 * t + pc, :],
                                    start=True,
                                    stop=True,
                                    tile_position=(64 * hh, 0),
                                )
                        for rec in completed:
                            flush_set(rec)

    nc.finalize()
    return nc


def _make_masks(g: int) -> np.ndarray:
    # mask[j][kk, qq] = 1 if query (512t + qq) >= key 128*(4t + 2j + g) + kk
    m = np.zeros((2, 128, QB), dtype=np.float32)
    for j in range(2):
        dk = 128 * (2 * j + g) + np.arange(128)[:, None]
        dq = np.arange(QB)[None, :]
        m[j] = (dq >= dk).astype(np.float32)
    return m.astype(BF16)


def _shard_inputs(input, Wq, Wk, Wv):
    x = np.asarray(input)
    wkq = np.concatenate([Wk, Wq], axis=1).astype(np.float32)  # [D, 128]
    # partition-major relayout: wkq_h[p, dc*128+j] = wkq[dc*128+p, j]
    wkq_h = np.ascontiguousarray(
        wkq.reshape(8, 128, 128).transpose(1, 0, 2).reshape(128, 8 * 128)
    ).astype(BF16)
    wv_h = np.ascontiguousarray(
        np.asarray(Wv, dtype=np.float32).reshape(8, 128, H).transpose(1, 0, 2).reshape(128, 8 * H)
    ).astype(BF16)
    masks = []
    for g in range(2):
        m = _make_masks(g)  # [2, 128, QB]
        masks.append(np.ascontiguousarray(m.transpose(1, 0, 2).reshape(128, 2 * QB)))
    in_maps = []
    for c in range(8):
        b, g = c // 2, c % 2
        xs = x[b].reshape(S // 128, 128, D)[g::2].reshape(RPC, D)
        xT = np.ascontiguousarray(xs.T).astype(BF16)
        in_maps.append({"x": xT, "wkq": wkq_h, "wv": wv_h, "mask": masks[g]})
    return in_maps


def _unshard(results):
    out = np.empty((B, S, H), dtype=np.float32)
    for b in range(B):
        merged = results[2 * b]["out"] + results[2 * b + 1]["out"]
        out[b] = (merged[:H] / merged[H : H + 1]).T
    return out


def _run(inputs, trace=False):
    from concourse.bass_utils import run_bass_kernel_spmd

    if "nc" not in _CACHE:
        _CACHE["nc"] = _build()
    nc = _CACHE["nc"]
    in_maps = _shard_inputs(**inputs)
    res = run_bass_kernel_spmd(nc, in_maps, core_ids=list(range(8)), trace=trace)
    out = _unshard(res.results)
    return out, res


def kernel(**inputs) -> np.ndarray:
    out, _ = _run(inputs, trace=False)
    return out


# revision 14
# speedup vs baseline: 1.4231x; 1.3363x over previous
"""Distributed causal attention head for TRN2 (8 NeuronCores), v6.

Problem: B=4, S=4096, D=1024, H=64 fp32.
  q,k,v = x @ W{q,k,v}; scores = q k^T / sqrt(H); causal softmax; out = P v.

Sharding (fully SPMD-uniform, one NEFF, NO collectives):
  - 4 batches x 2 cores per batch. Within a pair the KEY dimension is split
    by interleaved 128-row chunks: core g owns global key chunks {2i+g}.
  - Each core loads the FULL batch x^T [1024, 4096] bf16 (host pre-transposed)
    with the 128-col chunks PAIR-SWAP permuted for g=1, so own key chunks sit
    at even slots on every core -> all slicing is core-independent. Masks are
    built in permuted query order per core; the host un-permutes the g=1
    output columns, merges the pair (add), divides, and transposes.
  - Every measured collective (barrier/AllGather) could not execute before
    ~50-60us into the kernel, so q is simply computed from the full x instead
    of being gathered: the extra projection work hides under the x DMA.

Compute layout:
  - k|q packed projection (wkq = [Wk|Wq] -> k in psum rows 0:64, q in 64:128)
    processed in 512-col eighths, each immediately followed by its attention
    q-block so the strict-FIFO PE never waits on not-yet-loaded data.
  - V^T for own chunks via strided N=256 matmuls (wv stationary), transposed
    into natural layout with the DMA-xbar transpose engine (no PE transposes).
  - Scores transposed with 2x PE row tiling (64-contract): tile (0,0) does
    local chunks 0..t of q-block t, tile (64,0) chunks t+1..2t+1. kT/qT are
    duplicated into both SBUF partition halves via SBUF->SBUF DMA on the
    ScalarE DMA rings (separate from the bulk x loads on the SP rings).
  - Each score slot = one 2-bank PSUM set (2 chunks). exp: ScalarE for the
    T0 chunk; the T8 chunk uses a one-op DVE Schraudolph bit-trick
    (bf16 bits = uint16(score * A + B)) on 3 of every 4 sets, balancing
    ScalarE and VectorE. V is augmented with a ones column so the AV matmul
    also produces the softmax denominator (row 64 of [65, 512]).
"""

import sys

sys.path.insert(0, "/opt/trn_rl_repo")

import numpy as np
import ml_dtypes

B, S, D, H = 4, 4096, 1024, 64
QB = 512
NQB = S // QB           # 8 query blocks / projection eighths
NKC = S // 256          # 16 own key chunks per core
BF16 = ml_dtypes.bfloat16

_CACHE = {}


def _build():
    import concourse.bass as bass
    import concourse.mybir as mybir
    from concourse import bacc, tile
    from concourse.bass import ts

    f32 = mybir.dt.float32
    bf16 = mybir.dt.bfloat16
    u16 = mybir.dt.uint16
    Alu = mybir.AluOpType
    Act = mybir.ActivationFunctionType

    # Schraudolph exp-approximation constants for the DVE path:
    # bf16 bits of exp(s/8) ~= uint16(s * SCH_A + SCH_B)
    SCH_C = 486411
    SCH_A = 0.125 * float(1 << 23) / float(np.log(2.0)) / 65536.0
    SCH_B = float((127 << 23) - SCH_C) / 65536.0

    nc = bacc.Bacc(None, target_bir_lowering=False)

    x_ext = nc.declare_dram_parameter("x", [D, S], bf16, isOutput=False)
    wkq_ext = nc.declare_dram_parameter("wkq", [128, 8 * 128], bf16, isOutput=False)
    wv_ext = nc.declare_dram_parameter("wv", [128, 8 * H], bf16, isOutput=False)
    mask_ext = nc.declare_dram_parameter("mask", [128, 2 * QB], bf16, isOutput=False)
    out_ext = nc.declare_dram_parameter("out", [H + 1, S], f32, isOutput=True)

    with tile.TileContext(nc) as tc:
        with tc.tile_pool(name="persist", bufs=1) as persist:
            # --- persistent SBUF tensors ---
            xT = persist.tile([128, 8, S // 128, 128], bf16, tag="xT")
            wkq_sb = persist.tile([128, 8, 128], bf16, tag="wkq")
            wv_sb = persist.tile([128, 8, H], bf16, tag="wv")
            mask_sb = persist.tile([128, 2, QB], bf16, tag="mask")
            kT2 = persist.tile([128, NKC, 128], bf16, tag="kT2")
            qT2 = persist.tile([128, S], bf16, tag="qT2")
            vT = persist.tile([64, NKC, 128], bf16, tag="vT")
            v_all = persist.tile([128, NKC, H + 2], bf16, tag="v_all")
            zjunk = persist.tile([128, 8], f32, tag="zjunk")
            ejunk = persist.tile([128, 8], bf16, tag="ejunk")

            # preload the exp activation table set early (it costs ~2.7us)
            nc.vector.memset(zjunk[:], 0.0)
            nc.scalar.activation(ejunk[:], zjunk[:], Act.Exp)
            nc.vector.memset(v_all[:, :, H], 1.0)

            # small contiguous weight/mask loads, then bulk x per column quarter
            nc.sync.dma_start(out=wkq_sb[:], in_=wkq_ext[:])
            nc.sync.dma_start(out=wv_sb[:], in_=wv_ext[:])
            nc.sync.dma_start(out=mask_sb[:], in_=mask_ext[:])
            for qt in range(4):
                for dc in range(8):
                    nc.sync.dma_start(
                        out=xT[:, dc, 8 * qt : 8 * (qt + 1), :],
                        in_=x_ext[ts(dc, 128), ts(qt, S // 4)],
                    )

            with (
                tc.tile_pool(name="pj", bufs=1, space="PSUM") as pj_pool,
                tc.tile_pool(name="pv", bufs=1, space="PSUM") as pv_pool,
                tc.tile_pool(name="st", bufs=2, space="PSUM") as st_pool,
                tc.tile_pool(name="av", bufs=2, space="PSUM") as av_pool,
                tc.tile_pool(name="p", bufs=3) as p_pool,
                tc.tile_pool(name="o", bufs=3) as o_pool,
            ):
                set_idx = 0

                for t in range(NQB):
                    # ---- projection eighth t: x cols [512t, 512t+512) ----
                    kq_ps = pj_pool.tile([128, QB], f32, tag="kq")
                    for dc in range(8):
                        nc.tensor.matmul(
                            kq_ps[:],
                            lhsT=wkq_sb[:, dc, :],
                            rhs=xT[:, dc, 4 * t : 4 * t + 4, :],
                            start=(dc == 0),
                            stop=(dc == 7),
                        )
                    # q: psum rows 64:128 -> qT2 high half, then low-half dup
                    nc.vector.tensor_copy(qT2[64:128, ts(t, QB)], kq_ps[64:128, :])
                    nc.scalar.dma_start(
                        out=qT2[0:64, ts(t, QB)], in_=qT2[64:128, ts(t, QB)]
                    )
                    # k: own chunks sit at even slots (4t, 4t+2)
                    for j in range(2):
                        nc.any.tensor_copy(
                            kT2[0:64, 2 * t + j, :],
                            kq_ps[0:64, 2 * j * 128 : (2 * j + 1) * 128],
                        )
                    nc.scalar.dma_start(
                        out=kT2[64:128, 2 * t : 2 * t + 2, :],
                        in_=kT2[0:64, 2 * t : 2 * t + 2, :],
                    )
                    # natural-layout V for the two own chunks (x^T stationary)
                    for j in range(2):
                        v_ps = pv_pool.tile([128, H], f32, tag="v")
                        for dc in range(8):
                            nc.tensor.matmul(
                                v_ps[:],
                                lhsT=xT[:, dc, 4 * t + 2 * j, :],
                                rhs=wv_sb[:, dc, :],
                                start=(dc == 0),
                                stop=(dc == 7),
                            )
                        nc.any.tensor_copy(v_all[:, 2 * t + j, 0:H], v_ps[:])

                    # ---- attention q-block t ----
                    E = 2 * (t + 1)
                    av = av_pool.tile([H + 1, QB], f32, tag="av")
                    for s in range(t + 1):
                        c0, c1 = s, t + 1 + s
                        st = st_pool.tile([128, 2, QB], f32, tag="st")
                        p = p_pool.tile([128, 2, QB], bf16, tag="p")
                        for hh, cid in ((0, c0), (1, c1)):
                            nc.tensor.matmul(
                                st[:, hh, :],
                                lhsT=kT2[64 * hh : 64 * hh + 64, cid, :],
                                rhs=qT2[64 * hh : 64 * hh + 64, ts(t, QB)],
                                start=True,
                                stop=True,
                                tile_position=(64 * hh, 0),
                            )
                        if set_idx % 4 != 3:
                            # T8 chunk via the DVE bit-trick, T0 via ScalarE
                            nc.vector.tensor_scalar(
                                p[:, 1, :].bitcast(u16),
                                st[:, 1, :],
                                SCH_A,
                                SCH_B,
                                Alu.mult,
                                Alu.add,
                            )
                            nc.scalar.activation(
                                p[:, 0, :], st[:, 0, :], Act.Exp, scale=0.125
                            )
                        else:
                            nc.scalar.activation(
                                p[:, :, :], st[:, :, :], Act.Exp, scale=0.125
                            )
                        set_idx += 1
                        for hh, cid in ((0, c0), (1, c1)):
                            if cid >= E - 2:
                                nc.vector.tensor_tensor(
                                    p[:, hh, :],
                                    p[:, hh, :],
                                    mask_sb[:, cid - (E - 2), :],
                                    Alu.mult,
                                )
                        for hh, cid in ((0, c0), (1, c1)):
                            nc.tensor.matmul(
                                av[:],
                                lhsT=v_all[:, cid, 0 : H + 1],
                                rhs=p[:, hh, :],
                                start=(cid == 0),
                                stop=(cid == E - 1),
                            )
                    o = o_pool.tile([H + 1, QB], f32, tag="o")
                    nc.vector.tensor_copy(o[:], av[:])
                    nc.sync.dma_start(out=out_ext[:, ts(t, QB)], in_=o[:])

    nc.finalize()
    return nc


def _make_masks(g: int) -> np.ndarray:
    # mask[j][kk, qq]: qq is the PERMUTED block-local query col; its global
    # offset inside the block is qq_g. Key row kk belongs to own chunk with
    # in-block global offset 128*(2j+g).
    m = np.zeros((2, 128, QB), dtype=np.float32)
    qq = np.arange(QB)
    chunk4 = qq // 128
    if g == 1:
        chunk4 = chunk4 ^ 1
    qq_g = chunk4 * 128 + (qq % 128)
    for j in range(2):
        dk = 128 * (2 * j + g) + np.arange(128)[:, None]
        m[j] = (qq_g[None, :] >= dk).astype(np.float32)
    return m.astype(BF16)


def _perm_cols(a: np.ndarray, g: int) -> np.ndarray:
    """Pair-swap 128-col chunks along the last axis when g=1 (involution)."""
    if g == 0:
        return a
    shp = a.shape
    v = a.reshape(shp[:-1] + (shp[-1] // 256, 2, 128))
    return np.ascontiguousarray(v[..., ::-1, :].reshape(shp))


def _shard_inputs(input, Wq, Wk, Wv):
    x = np.asarray(input)
    wkq = np.concatenate([Wk, Wq], axis=1).astype(np.float32)  # [D, 128]
    wkq_h = np.ascontiguousarray(
        wkq.reshape(8, 128, 128).transpose(1, 0, 2).reshape(128, 8 * 128)
    ).astype(BF16)
    wv_h = np.ascontiguousarray(
        np.asarray(Wv, dtype=np.float32)
        .reshape(8, 128, H)
        .transpose(1, 0, 2)
        .reshape(128, 8 * H)
    ).astype(BF16)
    masks = []
    for g in range(2):
        m = _make_masks(g)  # [2, 128, QB]
        masks.append(np.ascontiguousarray(m.transpose(1, 0, 2).reshape(128, 2 * QB)))
    in_maps = []
    xTs = {}
    for b in range(B):
        xT = np.ascontiguousarray(x[b].T).astype(BF16)  # [D, S] global cols
        xTs[(b, 0)] = xT
        xTs[(b, 1)] = _perm_cols(xT, 1)
    for c in range(8):
        b, g = c // 2, c % 2
        in_maps.append(
            {"x": xTs[(b, g)], "wkq": wkq_h, "wv": wv_h, "mask": masks[g]}
        )
    return in_maps


def _unshard(results):
    out = np.empty((B, S, H), dtype=np.float32)
    for b in range(B):
        merged = results[2 * b]["out"] + _perm_cols(results[2 * b + 1]["out"], 1)
        out[b] = (merged[:H] / merged[H : H + 1]).T
    return out


def _run(inputs, trace=False):
    from concourse.bass_utils import run_bass_kernel_spmd

    if "nc" not in _CACHE:
        _CACHE["nc"] = _build()
    nc = _CACHE["nc"]
    in_maps = _shard_inputs(**inputs)
    res = run_bass_kernel_spmd(nc, in_maps, core_ids=list(range(8)), trace=trace)
    out = _unshard(res.results)
    return out, res


def kernel(**inputs) -> np.ndarray:
    out, _ = _run(inputs, trace=False)
    return out
